# revision 2
# baseline (speedup 1.0000x reference)
"""Trainium2 Bass kernel for nn_AttentionLayer (sparse_attention).

Reference computation (per batch b):
    q = wq @ x + bq          [8, N]     (1x1 conv, d=8, N=H*W=4096)
    k = wk @ x + bk          [8, N]
    v = wv @ x + bv          [64, N]
    energy = q^T k           [N, N]
    attn = softmax(energy, axis=-1)
    out = gamma * (v @ attn^T) + x

Sharding: data-parallel over batch; 8 batches -> 8 NeuronCores, one batch
element per core.  Weights replicated.  No collectives.

Per-core layout strategy (avoids all large transposes):
  - x_aug [65, 4096] sbuf, row 64 = ones (bias via augmented matmul).
  - q, k computed into [8(d), N] layout, replicated at partitions 0 and 32
    (supports 2x PE row tiling for the tiny-K energy matmul).
  - vT [N, 64] computed directly transposed (lhsT = x chunk), augmented with
    a 65th column of ones so the attention matmul also produces the softmax
    denominator s[i] (row 64 of the psum output).
  - energy computed transposed: eT[j, i] = sum_d k[d,j] q[d,i] -> psum
    [128(j), 512(i)]; exp on ScalarE psum->sbuf; out_psum[65, i] accumulated
    over all 32 j-blocks.  softmax max-subtraction is unnecessary (|e| < ~6).
  - normalize with 1/s broadcast over partitions via a K=1 PE matmul.
"""

import os
import sys

import numpy as np

sys.path.insert(0, "/opt/trn_rl_repo")

B, C, HH, WW = 8, 64, 64, 64
N = HH * WW  # 4096
D = 8  # qk channels
IC = 512  # i-chunk (queries per psum accumulation)
N_IC = N // IC  # 8
JB = 128  # j-block (keys per energy tile)
N_JB = N // JB  # 32

_CACHE = {}


def _build_program():
    import concourse.bass as bass
    import concourse.tile as tile
    from concourse import bacc, mybir
    from concourse.masks import make_identity

    f32 = mybir.dt.float32

    nc = bacc.Bacc(
        "TRN2", target_bir_lowering=False, debug=False, enable_asserts=False
    )

    x_d = nc.dram_tensor("x", [C, N], f32, kind="ExternalInput").ap()
    wq_d = nc.dram_tensor("wq", [D, C], f32, kind="ExternalInput").ap()
    bq_d = nc.dram_tensor("bq", [D], f32, kind="ExternalInput").ap()
    wk_d = nc.dram_tensor("wk", [D, C], f32, kind="ExternalInput").ap()
    bk_d = nc.dram_tensor("bk", [D], f32, kind="ExternalInput").ap()
    wv_d = nc.dram_tensor("wv", [C, C], f32, kind="ExternalInput").ap()
    bv_d = nc.dram_tensor("bv", [C], f32, kind="ExternalInput").ap()
    gamma_d = nc.dram_tensor("gamma", [1], f32, kind="ExternalInput").ap()
    y_d = nc.dram_tensor("y", [C, N], f32, kind="ExternalOutput").ap()

    EXP = mybir.ActivationFunctionType.Exp

    with tile.TileContext(nc) as tc:
        from contextlib import ExitStack

        with ExitStack() as ctx:
            consts = ctx.enter_context(tc.tile_pool(name="consts", bufs=1))
            bigs = ctx.enter_context(tc.tile_pool(name="bigs", bufs=1))
            work = ctx.enter_context(tc.tile_pool(name="work", bufs=4))
            ypool = ctx.enter_context(tc.tile_pool(name="ypool", bufs=3))
            small = ctx.enter_context(tc.tile_pool(name="small", bufs=2))

            psum_e = ctx.enter_context(
                tc.tile_pool(name="psum_e", bufs=4, space="PSUM")
            )
            psum_o = ctx.enter_context(
                tc.tile_pool(name="psum_o", bufs=2, space="PSUM")
            )
            psum_x = ctx.enter_context(
                tc.tile_pool(name="psum_x", bufs=2, space="PSUM")
            )

            # ---------------- constants / weights prep ----------------
            ident = consts.tile([C, C], f32)
            make_identity(nc, ident)

            ones = consts.tile([65, C], f32)
            nc.vector.memset(ones, 1.0)

            # warm the Exp activation table early so the ~2.7us table load
            # overlaps the prep phase
            warm = consts.tile([1, 8], f32)
            nc.scalar.activation(warm, ones[0:1, 0:8], EXP)

            gcol = consts.tile([65, 1], f32)
            nc.sync.dma_start(out=gcol, in_=gamma_d.to_broadcast([65, 1]))

            # x_aug: [65, N], row 64 = ones
            x_aug = bigs.tile([65, N], f32)
            nc.sync.dma_start(out=x_aug[0:C, :], in_=x_d)
            nc.vector.memset(x_aug[C : C + 1, :], 1.0)

            # raw weights
            wq_sb = consts.tile([D, C], f32)
            wk_sb = consts.tile([D, C], f32)
            wv_sb = consts.tile([C, C], f32)
            nc.sync.dma_start(out=wq_sb, in_=wq_d)
            nc.sync.dma_start(out=wk_sb, in_=wk_d)
            nc.sync.dma_start(out=wv_sb, in_=wv_d)

            # fold gamma into wv (and bv below): out = gamma*(v@attnT) + x
            nc.vector.tensor_scalar_mul(wv_sb, wv_sb, gcol[0:C])

            # wqT_rep / wkT_rep: [65, 64]; cols [0:8] and [32:40] hold the
            # transposed weights, row 64 holds the bias (matches x_aug ones
            # row).  Other columns zero.
            wqT = consts.tile([65, C], f32)
            wkT = consts.tile([65, C], f32)
            nc.vector.memset(wqT, 0.0)
            nc.vector.memset(wkT, 0.0)
            nc.sync.dma_start(out=wqT[C : C + 1, 0:D], in_=bq_d[None, :])
            nc.sync.dma_start(out=wqT[C : C + 1, 32 : 32 + D], in_=bq_d[None, :])
            nc.sync.dma_start(out=wkT[C : C + 1, 0:D], in_=bk_d[None, :])
            nc.sync.dma_start(out=wkT[C : C + 1, 32 : 32 + D], in_=bk_d[None, :])

            pt = psum_x.tile([C, D], f32, tag="px")
            nc.tensor.transpose(pt, wq_sb, ident[0:D, 0:D])
            nc.vector.tensor_copy(out=wqT[0:C, 0:D], in_=pt)
            nc.vector.tensor_copy(out=wqT[0:C, 32 : 32 + D], in_=pt)

            pt2 = psum_x.tile([C, D], f32, tag="px")
            nc.tensor.transpose(pt2, wk_sb, ident[0:D, 0:D])
            nc.vector.tensor_copy(out=wkT[0:C, 0:D], in_=pt2)
            nc.vector.tensor_copy(out=wkT[0:C, 32 : 32 + D], in_=pt2)

            # wvT_aug [65, 64]: rows 0:64 = (gamma*wv)^T, row 64 = gamma*bv
            wvT = consts.tile([65, C], f32)
            nc.sync.dma_start(out=wvT[C : C + 1, :], in_=bv_d[None, :])
            nc.vector.tensor_scalar_mul(
                wvT[C : C + 1, :], wvT[C : C + 1, :], gcol[C : C + 1]
            )
            pt3 = psum_x.tile([C, C], f32, tag="px")
            nc.tensor.transpose(pt3, wv_sb, ident)
            nc.vector.tensor_copy(out=wvT[0:C, :], in_=pt3)

            # ---------------- projections ----------------
            # qk_sb [40, 2, N]: [0:8]=q, replicated at [32:40]; plane 0 = q,
            # plane 1 = k.  (replication supports later PE row tiling)
            qk_sb = bigs.tile([40, 2, N], f32)
            for ic in range(N_IC):
                sl = slice(ic * IC, (ic + 1) * IC)
                pq = psum_x.tile([C, IC], f32, tag="px")
                nc.tensor.matmul(
                    pq, wqT, x_aug[:, sl], start=True, stop=True
                )
                nc.vector.tensor_copy(out=qk_sb[0:40, 0, sl], in_=pq[0:40, :])
                pk = psum_x.tile([C, IC], f32, tag="px")
                nc.tensor.matmul(
                    pk, wkT, x_aug[:, sl], start=True, stop=True
                )
                nc.vector.tensor_copy(out=qk_sb[0:40, 1, sl], in_=pk[0:40, :])

            # vT_aug [128, 32, 65]: vT_aug[p, jc, 0:64] = v^T[jc*128+p, :],
            # vT_aug[:, :, 64] = 1.0 (produces softmax denominator s)
            vT = bigs.tile([JB, N_JB, C + 1], f32)
            nc.vector.memset(vT[:, :, C : C + 1], 1.0)
            for jc8 in range(N_JB // 8):
                pv = psum_x.tile([JB, 8 * C], f32, tag="px")
                for j8 in range(8):
                    jc = jc8 * 8 + j8
                    nc.tensor.matmul(
                        pv[:, j8 * C : (j8 + 1) * C],
                        x_aug[:, jc * JB : (jc + 1) * JB],
                        wvT,
                        start=True,
                        stop=True,
                    )
                nc.vector.tensor_copy(
                    out=vT[:, jc8 * 8 : (jc8 + 1) * 8, 0:C],
                    in_=pv.rearrange("p (a b) -> p a b", a=8),
                )

            # ---------------- main attention loop ----------------
            for ic in range(N_IC):
                sl = slice(ic * IC, (ic + 1) * IC)
                out_ps = psum_o.tile([C + 1, IC], f32)
                for jb in range(N_JB):
                    e_ps = psum_e.tile([JB, IC], f32)
                    # eT[j, i] = sum_d k[d, j] * q[d, i]
                    nc.tensor.matmul(
                        e_ps,
                        qk_sb[0:D, 1, jb * JB : (jb + 1) * JB],
                        qk_sb[0:D, 0, sl],
                        start=True,
                        stop=True,
                    )
                    aT = work.tile([JB, IC], f32)
                    nc.scalar.activation(aT, e_ps, EXP)
                    # out_un[c, i] += sum_j vT[j, c] * aT[j, i]
                    # row 64 accumulates s[i] = sum_j aT[j, i]
                    nc.tensor.matmul(
                        out_ps,
                        vT[:, jb, :],
                        aT,
                        start=(jb == 0),
                        stop=(jb == N_JB - 1),
                    )

                # r = 1/s at partition 64
                r_sb = small.tile([C + 1, IC], f32)
                nc.vector.reciprocal(r_sb[C : C + 1, :], out_ps[C : C + 1, :])
                # broadcast r over 64 partitions via K=1 matmul
                rb_ps = psum_x.tile([C, IC], f32, tag="px")
                nc.tensor.matmul(
                    rb_ps,
                    ones[C : C + 1, 0:C],
                    r_sb[C : C + 1, :],
                    start=True,
                    stop=True,
                )
                rb_sb = small.tile([C, IC], f32)
                nc.vector.tensor_copy(out=rb_sb, in_=rb_ps)

                y_sb = ypool.tile([C, IC], f32)
                nc.vector.tensor_mul(y_sb, out_ps[0:C, :], rb_sb)
                nc.vector.tensor_add(y_sb, y_sb, x_aug[0:C, sl])
                nc.sync.dma_start(out=y_d[:, sl], in_=y_sb)

    nc.compile()
    return nc


def _get_program():
    if "nc" not in _CACHE:
        _CACHE["nc"] = _build_program()
    return _CACHE["nc"]


def kernel(**inputs) -> np.ndarray:
    nc = _get_program()
    from concourse.bass_utils import run_bass_kernel_spmd

    x = np.ascontiguousarray(np.asarray(inputs["x"], dtype=np.float32))
    shared = {
        k: np.ascontiguousarray(np.asarray(inputs[k], dtype=np.float32))
        for k in ("wq", "bq", "wk", "bk", "wv", "bv", "gamma")
    }
    in_maps = [
        {"x": x[b].reshape(C, N).copy(), **shared} for b in range(B)
    ]
    res = run_bass_kernel_spmd(nc, in_maps, list(range(B)))
    out = np.stack(
        [res.results[b]["y"].reshape(C, HH, WW) for b in range(B)], axis=0
    )
    return out.astype(np.float32)


if __name__ == "__main__":
    # smoke test with random data
    rng = np.random.default_rng(0)
    inputs = {
        "x": rng.standard_normal((B, C, HH, WW), dtype=np.float32),
        "wq": rng.standard_normal((D, C), dtype=np.float32) * 0.05,
        "bq": rng.standard_normal((D,), dtype=np.float32) * 0.05,
        "wk": rng.standard_normal((D, C), dtype=np.float32) * 0.05,
        "bk": rng.standard_normal((D,), dtype=np.float32) * 0.05,
        "wv": rng.standard_normal((C, C), dtype=np.float32) * 0.05,
        "bv": rng.standard_normal((C,), dtype=np.float32) * 0.05,
        "gamma": rng.standard_normal((1,), dtype=np.float32),
    }
    out = kernel(**inputs)
    print("out", out.shape, out.dtype, float(np.abs(out).max()))


# revision 5
# speedup vs baseline: 1.8576x; 1.8576x over previous
"""Trainium2 Bass kernel for nn_AttentionLayer (sparse_attention).

Reference computation (per batch b):
    q = wq @ x + bq          [8, N]     (1x1 conv, d=8, N=H*W=4096)
    k = wk @ x + bk          [8, N]
    v = wv @ x + bv          [64, N]
    energy = q^T k           [N, N]
    attn = softmax(energy, axis=-1)
    out = gamma * (v @ attn^T) + x

Sharding: data-parallel over batch; 8 batches -> 8 NeuronCores, one batch
element per core.  Weights replicated.  No collectives.

Per-core layout strategy (avoids all large transposes):
  - x_aug [65, 4096] sbuf, row 64 = ones (bias via augmented matmul); kept
    in f32 for the final residual add and in bf16 for matmul operands
    (fp32 matmuls lower to 2 PE passes at ~3x the cost - avoid).
  - q, k computed into [8(d), N] bf16 layout, replicated at partitions 0 and
    32 (supports 2x PE row tiling for the tiny-K energy matmul).
  - vT [N, 64] computed directly transposed (lhsT = x chunk), augmented with
    a 65th column of ones so the attention matmul also produces the softmax
    denominator s[i] (row 64 of the psum output).
  - energy computed transposed: eT[j, i] = sum_d k[d,j] q[d,i] -> psum
    [128(j), IC(i)] f32; exp on ScalarE psum->sbuf bf16; out_psum[65, i]
    accumulated over all 32 j-blocks.  softmax max-subtraction is
    unnecessary (|e| < ~6, exp is safe in f32).
  - normalize with 1/s broadcast over partitions via a K=1 PE matmul.

Accuracy note: the attention term is ~1% of the output magnitude (residual
dominates), so bf16 matmul operands cost ~1e-4 final relative error.
"""

import os
import sys

import numpy as np

sys.path.insert(0, "/opt/trn_rl_repo")

B, C, HH, WW = 8, 64, 64, 64
N = HH * WW  # 4096
D = 8  # qk channels
IC = 512  # i-chunk (queries per psum accumulation)
N_IC = N // IC  # 8
JB = 128  # j-block (keys per energy tile)
N_JB = N // JB  # 32

_CACHE = {}


def _build_program():
    import concourse.bass as bass
    import concourse.tile as tile
    from concourse import bacc, mybir
    from concourse.masks import make_identity

    f32 = mybir.dt.float32
    bf16 = mybir.dt.bfloat16

    nc = bacc.Bacc(
        "TRN2", target_bir_lowering=False, debug=False, enable_asserts=False
    )

    x_d = nc.dram_tensor("x", [C, N], f32, kind="ExternalInput").ap()
    wq_d = nc.dram_tensor("wq", [D, C], f32, kind="ExternalInput").ap()
    bq_d = nc.dram_tensor("bq", [D], f32, kind="ExternalInput").ap()
    wk_d = nc.dram_tensor("wk", [D, C], f32, kind="ExternalInput").ap()
    bk_d = nc.dram_tensor("bk", [D], f32, kind="ExternalInput").ap()
    wv_d = nc.dram_tensor("wv", [C, C], f32, kind="ExternalInput").ap()
    bv_d = nc.dram_tensor("bv", [C], f32, kind="ExternalInput").ap()
    gamma_d = nc.dram_tensor("gamma", [1], f32, kind="ExternalInput").ap()
    y_d = nc.dram_tensor("y", [C, N], f32, kind="ExternalOutput").ap()

    EXP = mybir.ActivationFunctionType.Exp

    with tile.TileContext(nc) as tc:
        from contextlib import ExitStack

        with ExitStack() as ctx:
            consts = ctx.enter_context(tc.tile_pool(name="consts", bufs=1))
            bigs = ctx.enter_context(tc.tile_pool(name="bigs", bufs=1))
            work = ctx.enter_context(tc.tile_pool(name="work", bufs=4))
            ypool = ctx.enter_context(tc.tile_pool(name="ypool", bufs=3))
            small = ctx.enter_context(tc.tile_pool(name="small", bufs=2))

            psum_e = ctx.enter_context(
                tc.tile_pool(name="psum_e", bufs=4, space="PSUM")
            )
            psum_o = ctx.enter_context(
                tc.tile_pool(name="psum_o", bufs=2, space="PSUM")
            )
            psum_x = ctx.enter_context(
                tc.tile_pool(name="psum_x", bufs=2, space="PSUM")
            )

            # ---------------- constants / weights prep ----------------
            ident = consts.tile([C, C], f32)
            make_identity(nc, ident)

            ones = consts.tile([65, C], bf16)
            nc.vector.memset(ones, 1.0)

            # warm the Exp activation table early so the ~2.7us table load
            # overlaps the prep phase
            warm = consts.tile([1, 8], f32)
            nc.scalar.activation(warm, ident[0:1, 0:8], EXP)

            gcol = consts.tile([65, 1], f32)
            nc.sync.dma_start(out=gcol, in_=gamma_d.to_broadcast([65, 1]))

            # x_aug: [65, N] f32, row 64 = ones; x_bf: bf16 copy for matmuls
            x_aug = bigs.tile([65, N], f32)
            nc.sync.dma_start(out=x_aug[0:C, :], in_=x_d)
            x_bf = bigs.tile([65, N], bf16)
            nc.vector.tensor_copy(out=x_bf[0:C, :], in_=x_aug[0:C, :])
            nc.vector.memset(x_bf[C : C + 1, :], 1.0)

            # raw weights
            wq_sb = consts.tile([D, C], f32)
            wk_sb = consts.tile([D, C], f32)
            wv_sb = consts.tile([C, C], f32)
            nc.sync.dma_start(out=wq_sb, in_=wq_d)
            nc.sync.dma_start(out=wk_sb, in_=wk_d)
            nc.sync.dma_start(out=wv_sb, in_=wv_d)

            # fold gamma into wv (and bv below): out = gamma*(v@attnT) + x
            nc.vector.tensor_scalar_mul(wv_sb, wv_sb, gcol[0:C])

            # wqT_rep / wkT_rep: [65, 64] bf16; cols [0:8] and [32:40] hold
            # the transposed weights, row 64 holds the bias (matches x ones
            # row).  Other columns zero.
            wqT = consts.tile([65, C], bf16)
            wkT = consts.tile([65, C], bf16)
            nc.vector.memset(wqT, 0.0)
            nc.vector.memset(wkT, 0.0)
            bst = consts.tile([1, 2 * D + C], f32)  # bias staging (f32 dma)
            nc.sync.dma_start(out=bst[:, 0:D], in_=bq_d[None, :])
            nc.sync.dma_start(out=bst[:, D : 2 * D], in_=bk_d[None, :])
            nc.sync.dma_start(out=bst[:, 2 * D :], in_=bv_d[None, :])

            pt = psum_x.tile([C, D], f32, tag="px")
            nc.tensor.transpose(pt, wq_sb, ident[0:D, 0:D])
            nc.vector.tensor_copy(out=wqT[0:C, 0:D], in_=pt)
            nc.vector.tensor_copy(out=wqT[0:C, 32 : 32 + D], in_=pt)

            pt2 = psum_x.tile([C, D], f32, tag="px")
            nc.tensor.transpose(pt2, wk_sb, ident[0:D, 0:D])
            nc.vector.tensor_copy(out=wkT[0:C, 0:D], in_=pt2)
            nc.vector.tensor_copy(out=wkT[0:C, 32 : 32 + D], in_=pt2)

            # bias rows: wqT/wkT live on partition 64; biases staged on
            # partition 0 -> move via small K=1 matmuls would be overkill;
            # stage biases at partition 64 directly with a second DMA.
            bst64 = consts.tile([65, 2 * D + C], f32)
            nc.sync.dma_start(out=bst64[C : C + 1, 0:D], in_=bq_d[None, :])
            nc.sync.dma_start(
                out=bst64[C : C + 1, D : 2 * D], in_=bk_d[None, :]
            )
            nc.sync.dma_start(out=bst64[C : C + 1, 2 * D :], in_=bv_d[None, :])
            nc.vector.tensor_copy(
                out=wqT[C : C + 1, 0:D], in_=bst64[C : C + 1, 0:D]
            )
            nc.vector.tensor_copy(
                out=wqT[C : C + 1, 32 : 32 + D], in_=bst64[C : C + 1, 0:D]
            )
            nc.vector.tensor_copy(
                out=wkT[C : C + 1, 0:D], in_=bst64[C : C + 1, D : 2 * D]
            )
            nc.vector.tensor_copy(
                out=wkT[C : C + 1, 32 : 32 + D], in_=bst64[C : C + 1, D : 2 * D]
            )

            # wvT_aug [65, 64] bf16: rows 0:64 = (gamma*wv)^T, row 64 = gamma*bv
            wvT = consts.tile([65, C], bf16)
            nc.vector.tensor_scalar_mul(
                bst64[C : C + 1, 2 * D :], bst64[C : C + 1, 2 * D :],
                gcol[C : C + 1],
            )
            nc.vector.tensor_copy(
                out=wvT[C : C + 1, :], in_=bst64[C : C + 1, 2 * D :]
            )
            pt3 = psum_x.tile([C, C], f32, tag="px")
            nc.tensor.transpose(pt3, wv_sb, ident)
            nc.vector.tensor_copy(out=wvT[0:C, :], in_=pt3)

            # ---------------- projections ----------------
            # qk_sb [40, 2, N] bf16: [0:8]=q, replicated at [32:40]; plane 0
            # = q, plane 1 = k.  (replication supports PE row tiling)
            qk_sb = bigs.tile([40, 2, N], bf16)
            for ic in range(N_IC):
                sl = slice(ic * IC, (ic + 1) * IC)
                pq = psum_x.tile([C, IC], f32, tag="px")
                nc.tensor.matmul(pq, wqT, x_bf[:, sl], start=True, stop=True)
                nc.vector.tensor_copy(out=qk_sb[0:40, 0, sl], in_=pq[0:40, :])
                pk = psum_x.tile([C, IC], f32, tag="px")
                nc.tensor.matmul(pk, wkT, x_bf[:, sl], start=True, stop=True)
                nc.vector.tensor_copy(out=qk_sb[0:40, 1, sl], in_=pk[0:40, :])

            # vT_aug [128, 32, 65] bf16: vT[p, jc, 0:64] = v^T[jc*128+p, :],
            # vT[:, :, 64] = 1.0 (produces softmax denominator s)
            vT = bigs.tile([JB, N_JB, C + 1], bf16)
            nc.vector.memset(vT[:, :, C : C + 1], 1.0)
            for jc8 in range(N_JB // 8):
                pv = psum_x.tile([JB, 8 * C], f32, tag="px")
                for j8 in range(8):
                    jc = jc8 * 8 + j8
                    nc.tensor.matmul(
                        pv[:, j8 * C : (j8 + 1) * C],
                        x_bf[:, jc * JB : (jc + 1) * JB],
                        wvT,
                        start=True,
                        stop=True,
                    )
                nc.vector.tensor_copy(
                    out=vT[:, jc8 * 8 : (jc8 + 1) * 8, 0:C],
                    in_=pv.rearrange("p (a b) -> p a b", a=8),
                )

            # ---------------- main attention loop ----------------
            for ic in range(N_IC):
                sl = slice(ic * IC, (ic + 1) * IC)
                out_ps = psum_o.tile([C + 1, IC], f32)
                for jb in range(N_JB):
                    e_ps = psum_e.tile([JB, IC], f32)
                    # eT[j, i] = sum_d k[d, j] * q[d, i]
                    nc.tensor.matmul(
                        e_ps,
                        qk_sb[0:D, 1, jb * JB : (jb + 1) * JB],
                        qk_sb[0:D, 0, sl],
                        start=True,
                        stop=True,
                    )
                    aT = work.tile([JB, IC], bf16)
                    nc.scalar.activation(aT, e_ps, EXP)
                    # out_un[c, i] += sum_j vT[j, c] * aT[j, i]
                    # row 64 accumulates s[i] = sum_j aT[j, i]
                    nc.tensor.matmul(
                        out_ps,
                        vT[:, jb, :],
                        aT,
                        start=(jb == 0),
                        stop=(jb == N_JB - 1),
                    )

                # r = 1/s at partition 64 (~18 bit accurate; plenty for the
                # softmax denominator), then bf16 for the broadcast matmul
                r_sb = small.tile([C + 1, IC], f32)
                nc.vector.reciprocal(
                    r_sb[C : C + 1, :], out_ps[C : C + 1, :]
                )
                r_bf = small.tile([C + 1, IC], bf16)
                nc.vector.tensor_copy(
                    out=r_bf[C : C + 1, :], in_=r_sb[C : C + 1, :]
                )
                # broadcast r over 64 partitions via K=1 matmul
                rb_ps = psum_x.tile([C, IC], f32, tag="px")
                nc.tensor.matmul(
                    rb_ps,
                    ones[C : C + 1, 0:C],
                    r_bf[C : C + 1, :],
                    start=True,
                    stop=True,
                )
                rb_sb = small.tile([C, IC], f32)
                nc.vector.tensor_copy(out=rb_sb, in_=rb_ps)

                y_sb = ypool.tile([C, IC], f32)
                nc.vector.tensor_mul(y_sb, out_ps[0:C, :], rb_sb)
                nc.vector.tensor_add(y_sb, y_sb, x_aug[0:C, sl])
                nc.sync.dma_start(out=y_d[:, sl], in_=y_sb)

    nc.compile()
    return nc


def _get_program():
    if "nc" not in _CACHE:
        _CACHE["nc"] = _build_program()
    return _CACHE["nc"]


def kernel(**inputs) -> np.ndarray:
    nc = _get_program()
    from concourse.bass_utils import run_bass_kernel_spmd

    x = np.ascontiguousarray(np.asarray(inputs["x"], dtype=np.float32))
    shared = {
        k: np.ascontiguousarray(np.asarray(inputs[k], dtype=np.float32))
        for k in ("wq", "bq", "wk", "bk", "wv", "bv", "gamma")
    }
    in_maps = [
        {"x": x[b].reshape(C, N).copy(), **shared} for b in range(B)
    ]
    res = run_bass_kernel_spmd(nc, in_maps, list(range(B)))
    out = np.stack(
        [res.results[b]["y"].reshape(C, HH, WW) for b in range(B)], axis=0
    )
    return out.astype(np.float32)


if __name__ == "__main__":
    rng = np.random.default_rng(0)
    inputs = {
        "x": rng.standard_normal((B, C, HH, WW), dtype=np.float32),
        "wq": rng.standard_normal((D, C), dtype=np.float32) * 0.05,
        "bq": rng.standard_normal((D,), dtype=np.float32) * 0.05,
        "wk": rng.standard_normal((D, C), dtype=np.float32) * 0.05,
        "bk": rng.standard_normal((D,), dtype=np.float32) * 0.05,
        "wv": rng.standard_normal((C, C), dtype=np.float32) * 0.05,
        "bv": rng.standard_normal((C,), dtype=np.float32) * 0.05,
        "gamma": rng.standard_normal((1,), dtype=np.float32),
    }
    out = kernel(**inputs)
    print("out", out.shape, out.dtype, float(np.abs(out).max()))
